# revision 1
# baseline (speedup 1.0000x reference)
"""Trainium2 Bass kernel for the vq_codebook classifier problem.

Computes, for X [4096, 512] f32 and grp [1, 512, 100] f32:
    l1   = sum_d |X[n,d] - grp[0,d,c]|            -> [N, C]
    norm = softmax(-l1, axis=1)
    cs   = (X @ g) / max(|X| * |g|, eps)           (cosine similarity)
    out  = max_c(cs) * softmax(cs, axis=1) * norm

Sharding: data-parallel over N across 8 NeuronCores (512 rows each),
grp replicated.

Math notes used by the kernel:
  |v| = 2*relu(v) - v, so
  l1[n,c] = 2*sum_d relu(x-g) - sum_d x + sum_d g
The "sum_d x" term is constant over classes and cancels inside
softmax(-l1), so it is dropped. sum_d g (G1) is injected into the same
PSUM accumulator via a rank-1 matmul (ones-row x 0.5*G1).

relu(x - g) tiles are produced in a d-on-partition layout by the vector
engine (tensor_scalar sub+max, fp32 2x mode) and the scalar engine
(activation Relu with per-partition bias -g). The partition (d)
reduction runs on TensorE with the relu tile as the *stationary*
operand and a ones-column as the moving operand, so each class lands in
one PSUM *column* of an [n, c] accumulator (PE outputs may only start
at partition 0/32/64, but free-dim offsets are unrestricted. This also
leaves results pre-transposed for the per-row epilogue). float32r keeps
the weight load at full rate; the reduction stays exact because the
ones operand splits into bf16 parts losslessly.
"""

import numpy as np

P = 128
R = 512          # rows per core (4096 / 8 cores)
D = 512
C = 100
NT = D // P      # 4 d-tiles
RT = R // P      # 4 row-tiles
N_CORES = 8

_CACHE = {}


def _split_excess_waits(nc, limit=1):
    """walrus in this container rejects instructions carrying more than
    one sync wait ("Too many sync wait commands"). Hoist excess waits
    onto same-engine NoOps inserted immediately before the instruction."""
    import concourse.mybir as mb
    import bass_rust

    n_id = [0]

    def mknop(engine, waits):
        n_id[0] += 1
        return bass_rust.InstNoOp(
            name=f"waitsplit-{n_id[0]}", engine=engine, ins=[], outs=[],
            sync_info=mb.SyncInfo(on_wait=list(waits), on_update=[]),
        )

    for fn in nc.m.functions:
        for bb in fn.blocks:
            insts = bb.instructions
            out = []
            for inst in insts:
                si = inst.sync_info
                if si is not None and si.on_wait and len(si.on_wait) > limit:
                    waits = list(si.on_wait)
                    extra, keep = waits[:-limit], waits[-limit:]
                    for w in extra:
                        out.append(mknop(inst.engine, [w]))
                    inst.sync_info = mb.SyncInfo(
                        on_wait=keep, on_update=list(si.on_update)
                    )
                out.append(inst)
            insts[:] = out


def _build_nc(reps: int = 1):
    import concourse.bass as bass
    import concourse.mybir as mybir
    import concourse.tile as tile
    from concourse.masks import make_identity
    from contextlib import ExitStack

    f32 = mybir.dt.float32
    f16 = mybir.dt.float16
    Alu = mybir.AluOpType
    Act = mybir.ActivationFunctionType
    Ax = mybir.AxisListType

    nc = bass.Bass(target_bir_lowering=False)
    Xd = nc.declare_dram_parameter("X", [R, D], f32, isOutput=False)
    Gd = nc.declare_dram_parameter("G", [D, C], f32, isOutput=False)
    Yd = nc.declare_dram_parameter("Y", [R, C], f32, isOutput=True)

    with ExitStack() as ctx:
        tc = ctx.enter_context(tile.TileContext(nc))
        consts = ctx.enter_context(tc.tile_pool(name="consts", bufs=1))
        xr_pool = ctx.enter_context(tc.tile_pool(name="xr", bufs=RT))
        xt_pool = ctx.enter_context(tc.tile_pool(name="xt", bufs=NT))
        g_pool = ctx.enter_context(tc.tile_pool(name="g", bufs=NT))
        gnb_pool = ctx.enter_context(tc.tile_pool(name="gnb", bufs=NT))
        gh_pool = ctx.enter_context(tc.tile_pool(name="gh", bufs=NT))
        small = ctx.enter_context(tc.tile_pool(name="small", bufs=24))
        scratch = ctx.enter_context(tc.tile_pool(name="scratch", bufs=2))
        u_pool = ctx.enter_context(tc.tile_pool(name="u", bufs=10))
        out_pool = ctx.enter_context(tc.tile_pool(name="out", bufs=RT))

        for _rep in range(reps):
            # ---- constants ----
            ident = consts.tile([P, P], f32)
            make_identity(nc, ident[:])
            ones_col = consts.tile([P, 1], f16)
            nc.vector.memset(ones_col[:], 1.0)
            ones_row = consts.tile([1, P], f32)
            nc.vector.memset(ones_row[:], 1.0)

            # ---- load inputs ----
            xr = []
            for k in range(RT):
                t = xr_pool.tile([P, D], f32, tag="xr", name=f"xr{k}")
                nc.sync.dma_start(t[:], Xd[k * P:(k + 1) * P, :])
                xr.append(t)
            g = []
            for t_ in range(NT):
                gt = g_pool.tile([P, C], f32, tag="g", name=f"g{t_}")
                nc.sync.dma_start(gt[:], Gd[t_ * P:(t_ + 1) * P, :])
                g.append(gt)

            xt = [xt_pool.tile([P, R], f16, tag="xt", name=f"xt{i}") for i in range(NT)]
            gT = consts.tile([C, D], f32)
            gh = [gh_pool.tile([P, C], f16, tag="gh", name=f"gh{i}") for i in range(NT)]
            g1row = consts.tile([1, C], f32)

            # prep-phase PSUM transposes live in their own pool so the banks
            # are free again before the 8 accumulator banks are allocated
            with tc.tile_pool(name="tp_ps", bufs=2, space="PSUM") as tp_ps:
                # ---- X^T tiles (d on partitions) via PE transpose ----
                for dt in range(NT):
                    for rt in range(RT):
                        tp = tp_ps.tile([P, P], f32, tag="tp")
                        nc.tensor.transpose(
                            tp[:], xr[rt][:, dt * P:(dt + 1) * P], ident[:]
                        )
                        if (dt + rt) % 2 == 0:
                            nc.vector.tensor_copy(xt[dt][:, rt * P:(rt + 1) * P], tp[:])
                        else:
                            nc.scalar.copy(xt[dt][:, rt * P:(rt + 1) * P], tp[:])

                # ---- G^T [100, 512] ----
                for t_ in range(NT):
                    tp = tp_ps.tile([C, P], f32, tag="tpg")
                    nc.tensor.transpose(tp[:], g[t_][:], ident[:])
                    nc.vector.tensor_copy(gT[:, t_ * P:(t_ + 1) * P], tp[:])

                # column norms and sums of g
                gsq = scratch.tile([C, D], f32, tag="gsq")
                nc.vector.tensor_tensor(gsq[:], gT[:], gT[:], Alu.mult)
                gn2 = small.tile([C, 1], f32, tag="gn2")
                nc.vector.tensor_reduce(gn2[:], gsq[:], Ax.X, Alu.add)
                gn = small.tile([C, 1], f32, tag="gn")
                nc.scalar.activation(gn[:], gn2[:], Act.Sqrt)
                rgn = small.tile([C, 1], f32, tag="rgn")
                nc.vector.reciprocal(rgn[:], gn[:])
                g1 = small.tile([C, 1], f32, tag="g1")
                nc.vector.tensor_reduce(g1[:], gT[:], Ax.X, Alu.add)
                g1h = small.tile([C, 1], f32, tag="g1h")
                nc.vector.tensor_scalar_mul(g1h[:], g1[:], 0.5)

                # ghat = g * (1/|g|) per column, back in [d, c] layout
                ghT = scratch.tile([C, D], f32, tag="ghT")
                nc.vector.tensor_scalar_mul(ghT[:], gT[:], rgn[:])
                for t_ in range(NT):
                    tp = tp_ps.tile([P, C], f32, tag="tpg2")
                    nc.tensor.transpose(
                        tp[:], ghT[:, t_ * P:(t_ + 1) * P], ident[:C, :C]
                    )
                    nc.vector.tensor_copy(gh[t_][:], tp[:])

                # G1/2 as a row [1, C] for the rank-1 inject
                g1row_ps = tp_ps.tile([1, C], f32, tag="tpg3")
                nc.tensor.transpose(g1row_ps[:], g1h[:], ident[:C, :C])
                nc.vector.tensor_copy(g1row[:], g1row_ps[:])

            # ---- row norms 1/|x| (rows layout) ----
            rxn = []
            for k in range(RT):
                sq = scratch.tile([P, D], f32, tag="sq")
                xn2 = small.tile([P, 1], f32, tag="xn2", name=f"xn2_{k}")
                nc.scalar.activation(sq[:], xr[k][:], Act.Square, accum_out=xn2[:])
                xn = small.tile([P, 1], f32, tag="xn", name=f"xn{k}")
                nc.scalar.activation(xn[:], xn2[:], Act.Sqrt)
                r = small.tile([P, 1], f32, tag="rxn", name=f"rxn{k}")
                nc.vector.reciprocal(r[:], xn[:])
                rxn.append(r)

            # -G tiles for the scalar-engine relu bias
            gneg = [gnb_pool.tile([P, C], f32, tag="gneg", name=f"gneg{i}")
                    for i in range(NT)]
            for t_ in range(NT):
                nc.vector.tensor_scalar_mul(gneg[t_][:], g[t_][:], -1.0)

            # ---- accumulators: [n, c] per row-tile, 8 PSUM banks total ----
            with (
                tc.tile_pool(name="s_ps", bufs=RT, space="PSUM") as s_pool,
                tc.tile_pool(name="d_ps", bufs=RT, space="PSUM") as d_pool,
            ):
                s_ps = [s_pool.tile([P, C], f32, tag="s", name=f"s{k}")
                        for k in range(RT)]
                dot_ps = [d_pool.tile([P, C], f32, tag="d", name=f"d{k}")
                          for k in range(RT)]

                # cosine: DOT[n, c] = sum_d xT[d, n] * ghat[d, c]
                for k in range(RT):
                    for t_ in range(NT):
                        nc.tensor.matmul(
                            dot_ps[k][:],
                            lhsT=xt[t_][:, k * P:(k + 1) * P],
                            rhs=gh[t_][:],
                            start=(t_ == 0),
                            stop=(t_ == NT - 1),
                        )

                # S[n, c] = sum_d relu(x - g) + 0.5*G1[c] (broadcast inject)
                for k in range(RT):
                    nc.tensor.matmul(
                        s_ps[k][:],
                        lhsT=ones_row[:],
                        rhs=g1row[:],
                        start=True,
                        stop=False,
                    )
                for c in range(C):
                    for t_ in range(NT):
                        u = u_pool.tile([P, R], f16, tag="u")
                        if (c * NT + t_) % 3 == 2:
                            nc.scalar.activation(
                                u[:], xt[t_][:], Act.Relu,
                                bias=gneg[t_][:, c:c + 1], scale=1.0,
                            )
                        else:
                            nc.vector.tensor_scalar(
                                u[:], xt[t_][:], g[t_][:, c:c + 1], 0.0,
                                Alu.subtract, Alu.max,
                            )
                        last = (c == C - 1) and (t_ == NT - 1)
                        for k in range(RT):
                            nc.tensor.matmul(
                                s_ps[k][:, c:c + 1],
                                lhsT=u[:, k * P:(k + 1) * P],
                                rhs=ones_col[:],
                                start=False,
                                stop=last,
                            )

                # ---- epilogue per row-tile ----
                for k in range(RT):
                    # cs = dot * (1/|x|)  (1/|g| already folded into ghat)
                    cs = scratch.tile([P, C], f32, tag="cs")
                    nc.vector.tensor_scalar_mul(cs[:], dot_ps[k][:], rxn[k][:])
                    conf = small.tile([P, 1], f32, tag="conf")
                    nc.vector.tensor_reduce(conf[:], cs[:], Ax.X, Alu.max)
                    # confusion = softmax(cs): cs in [-1, 1], no shift needed
                    e2 = scratch.tile([P, C], f32, tag="e2")
                    s2 = small.tile([P, 1], f32, tag="s2")
                    nc.scalar.activation(e2[:], cs[:], Act.Exp, accum_out=s2[:])
                    # norm = softmax(-l1), l1 = 2*S (+ row-constant, dropped)
                    m = small.tile([P, 1], f32, tag="m")
                    nc.vector.tensor_reduce(m[:], s_ps[k][:], Ax.X, Alu.min)
                    m2 = small.tile([P, 1], f32, tag="m2")
                    nc.vector.tensor_scalar_mul(m2[:], m[:], 2.0)
                    e1 = scratch.tile([P, C], f32, tag="e1")
                    s1 = small.tile([P, 1], f32, tag="s1")
                    nc.scalar.activation(
                        e1[:], s_ps[k][:], Act.Exp, bias=m2[:], scale=-2.0,
                        accum_out=s1[:],
                    )
                    # out = conf * (e1/s1) * (e2/s2) = (e1*e2) * (conf/(s1*s2))
                    den = small.tile([P, 1], f32, tag="den")
                    nc.vector.tensor_tensor(den[:], s1[:], s2[:], Alu.mult)
                    rden = small.tile([P, 1], f32, tag="rden")
                    nc.vector.reciprocal(rden[:], den[:])
                    fac = small.tile([P, 1], f32, tag="fac")
                    nc.vector.tensor_tensor(fac[:], conf[:], rden[:], Alu.mult)
                    prod = scratch.tile([P, C], f32, tag="prod")
                    nc.vector.tensor_tensor(prod[:], e1[:], e2[:], Alu.mult)
                    out_t = out_pool.tile([P, C], f32, tag="out")
                    nc.vector.tensor_scalar_mul(out_t[:], prod[:], fac[:])
                    nc.sync.dma_start(Yd[k * P:(k + 1) * P, :], out_t[:])

    _split_excess_waits(nc)
    return nc


def kernel(X: np.ndarray, grp: np.ndarray) -> np.ndarray:
    from concourse.bass_utils import run_bass_kernel_spmd

    if "nc" not in _CACHE:
        _CACHE["nc"] = _build_nc()
    nc = _CACHE["nc"]

    X = np.ascontiguousarray(X, dtype=np.float32)
    g2d = np.ascontiguousarray(grp.reshape(D, C), dtype=np.float32)
    shards = np.split(X, N_CORES, axis=0)
    in_maps = [{"X": s, "G": g2d} for s in shards]
    last_err = None
    for _attempt in range(3):
        try:
            res = run_bass_kernel_spmd(nc, in_maps, list(range(N_CORES)))
            break
        except Exception as e:  # transient device/tunnel hiccups
            last_err = e
            import time
            time.sleep(2.0)
    else:
        raise last_err
    out = np.concatenate(
        [res.results[i]["Y"] for i in range(N_CORES)], axis=0
    )
    return np.ascontiguousarray(out, dtype=np.float32)



# revision 5
# speedup vs baseline: 69.4889x; 69.4889x over previous
"""Trainium2 Bass kernel for the vq_codebook classifier problem.

Computes, for X [4096, 512] f32 and grp [1, 512, 100] f32:
    l1   = sum_d |X[n,d] - grp[0,d,c]|             -> [N, C]
    norm = softmax(-l1, axis=1)
    cs   = (X @ g) / max(|X| * |g|, eps)           (cosine similarity)
    out  = max_c(cs) * softmax(cs, axis=1) * norm

Sharding: data-parallel over N across 8 NeuronCores (512 rows each),
grp replicated.

This environment pays a large fixed cost PER INSTRUCTION (measured
~30-50us each, regardless of the work an instruction does), so the
kernel is built around a minimal instruction count with huge access
patterns:

  * The L1-distance tensor |x[n,d] - g[d,c]| for a block of 25 classes
    x all 512 rows is produced by ONE vector-engine tensor_tensor
    subtract over a [128, 4, 25, 512] access pattern (x broadcast over
    the class dim with a stride-0 AP; the prototype table broadcast
    over the row-tile dim).  The d-reduction with |.| is ONE
    tensor_reduce(axis=X, apply_absolute_value=True).  12 instructions
    total for the whole 26M-element L1 computation.
  * The prototype table is replicated across all 128 partitions by a
    partition-broadcast DMA directly from DRAM (c-major layout prepped
    host-side; layout transforms are part of the sharding contract).
  * The cosine GEMM runs transposed ([c, n] = g16^T-stationary x
    X^T-moving, 4 matmuls) and is transposed back to [n, c] by 4 PE
    transposes; 1/|g| is folded in before the transpose, 1/|x| after,
    each as one big broadcasted tensor_tensor.
  * The softmax/softmin/confidence epilogue is fused across all 4
    row-tiles with [128, 4, 100] access patterns (~14 instructions).
"""

import numpy as np

P = 128
R = 512          # rows per core (4096 / 8 cores)
D = 512
C = 100
RT = R // P      # 4 row-tiles
NT = D // P      # 4 d-tiles
CB = 25          # classes per L1 block
NCB = C // CB    # 4 class blocks
N_CORES = 8

_CACHE = {}


def _split_excess_waits(nc, limit=1):
    """walrus in this container rejects instructions carrying more than
    one sync wait ("Too many sync wait commands"). Hoist excess waits
    onto same-engine NoOps inserted immediately before the instruction."""
    import concourse.mybir as mb
    import bass_rust

    n_id = [0]

    def mknop(engine, waits):
        n_id[0] += 1
        return bass_rust.InstNoOp(
            name=f"waitsplit-{n_id[0]}", engine=engine, ins=[], outs=[],
            sync_info=mb.SyncInfo(on_wait=list(waits), on_update=[]),
        )

    for fn in nc.m.functions:
        for bb in fn.blocks:
            insts = bb.instructions
            out = []
            for inst in insts:
                si = inst.sync_info
                if si is not None and si.on_wait and len(si.on_wait) > limit:
                    waits = list(si.on_wait)
                    extra, keep = waits[:-limit], waits[-limit:]
                    for w in extra:
                        out.append(mknop(inst.engine, [w]))
                    inst.sync_info = mb.SyncInfo(
                        on_wait=keep, on_update=list(si.on_update)
                    )
                out.append(inst)
            insts[:] = out
    return nc


def _build_nc(reps: int = 1):
    import concourse.bass as bass
    import concourse.mybir as mybir
    import concourse.tile as tile
    from concourse.masks import make_identity
    from contextlib import ExitStack

    f32 = mybir.dt.float32
    f16 = mybir.dt.float16
    Alu = mybir.AluOpType
    Act = mybir.ActivationFunctionType
    Ax = mybir.AxisListType

    nc = bass.Bass(target_bir_lowering=False)
    X16d = nc.declare_dram_parameter("X16", [R, D], f16, isOutput=False)
    XT16d = nc.declare_dram_parameter("XT16", [D, R], f16, isOutput=False)
    GD16d = nc.declare_dram_parameter("GD16", [D, C], f16, isOutput=False)
    GT32d = nc.declare_dram_parameter("GT32", [C, D], f32, isOutput=False)
    GR16d = nc.declare_dram_parameter("GR16", [1, C * D], f16, isOutput=False)
    Yd = nc.declare_dram_parameter("Y", [R, C], f32, isOutput=True)

    with ExitStack() as ctx:
        tc = ctx.enter_context(tile.TileContext(nc))
        consts = ctx.enter_context(tc.tile_pool(name="consts", bufs=1))
        inp = ctx.enter_context(tc.tile_pool(name="inp", bufs=1))
        grep_pool = ctx.enter_context(tc.tile_pool(name="grep", bufs=2))
        dpool = ctx.enter_context(tc.tile_pool(name="dpool", bufs=1))
        work = ctx.enter_context(tc.tile_pool(name="work", bufs=1))
        small = ctx.enter_context(tc.tile_pool(name="small", bufs=2))

        for _rep in range(reps):
            ident = consts.tile([P, P], f32)
            make_identity(nc, ident[:])

            # ---- inputs ----
            x16 = inp.tile([P, RT, D], f16, tag="x16")
            nc.sync.dma_start(
                x16[:], X16d[:].rearrange("(k p) d -> p k d", p=P))
            xt16 = inp.tile([P, NT, R], f16, tag="xt16")
            nc.sync.dma_start(
                xt16[:], XT16d[:].rearrange("(t p) n -> p t n", p=P))
            g16 = inp.tile([P, NT, C], f16, tag="g16")
            nc.sync.dma_start(
                g16[:], GD16d[:].rearrange("(t p) c -> p t c", p=P))
            gt32 = inp.tile([C, D], f32, tag="gt32")
            nc.sync.dma_start(gt32[:], GT32d[:])

            # ---- row norms: rxn = 1/|x|  [128, RT] ----
            xsq = work.tile([P, RT, D], f32, tag="xsq")
            nc.vector.tensor_tensor(xsq[:], x16[:], x16[:], Alu.mult)
            xn2 = small.tile([P, RT], f32, tag="xn2")
            nc.vector.tensor_reduce(xn2[:], xsq[:], Ax.X, Alu.add)
            xn = small.tile([P, RT], f32, tag="xn")
            nc.scalar.activation(xn[:], xn2[:], Act.Sqrt)
            rxn = small.tile([P, RT], f32, tag="rxn")
            nc.vector.reciprocal(rxn[:], xn[:])

            # ---- prototype norms: rgn = 1/|g|  [C, 1] ----
            junk = work.tile([C, D], f32, tag="junk")
            gn2 = small.tile([C, 1], f32, tag="gn2")
            nc.scalar.activation(junk[:], gt32[:], Act.Square,
                                 accum_out=gn2[:])
            gn = small.tile([C, 1], f32, tag="gn")
            nc.scalar.activation(gn[:], gn2[:], Act.Sqrt)
            rgn = small.tile([C, 1], f32, tag="rgn")
            nc.vector.reciprocal(rgn[:], gn[:])

            # ---- cosine GEMM, transposed: dotT[c, n] ----
            with tc.tile_pool(name="ps", bufs=2, space="PSUM") as psp:
                dotT = psp.tile([C, R], f32, tag="dotT")
                for t in range(NT):
                    nc.tensor.matmul(
                        dotT[:], lhsT=g16[:, t, :], rhs=xt16[:, t, :],
                        start=(t == 0), stop=(t == NT - 1))
                # fold 1/|g| (per class = per partition here)
                csT = work.tile([C, R], f32, tag="csT")
                nc.vector.tensor_tensor(
                    csT[:], dotT[:], rgn[:].broadcast_to([C, R]), Alu.mult)
                # transpose back to [n, c] layout: cst [128, RT, C] (PSUM)
                cst = psp.tile([P, RT, C], f32, tag="cst")
                for k in range(RT):
                    nc.tensor.transpose(
                        cst[:, k, :], csT[:, k * P:(k + 1) * P],
                        ident[:C, :C])

                # ---- L1 distances: l1[n-part, k, c] ----
                l1 = work.tile([P, RT, C], f32, tag="l1")
                for cb in range(NCB):
                    grepb = grep_pool.tile([P, CB * D], f16, tag="grep")
                    nc.sync.dma_start(
                        grepb[:],
                        GR16d[0:1, cb * CB * D:(cb + 1) * CB * D]
                        .partition_broadcast(P))
                    diff = dpool.tile([P, RT, CB, D], f16, tag="diff")
                    nc.vector.tensor_tensor(
                        diff[:],
                        x16[:].unsqueeze(2).broadcast_to([P, RT, CB, D]),
                        grepb[:].rearrange("p (c d) -> p c d", c=CB)
                        .unsqueeze(1).broadcast_to([P, RT, CB, D]),
                        Alu.subtract)
                    nc.vector.tensor_reduce(
                        l1[:, :, cb * CB:(cb + 1) * CB], diff[:], Ax.X,
                        Alu.add, apply_absolute_value=True)

                # ---- epilogue, fused over all row-tiles ----
                # cs = dotT^T * rxn ; conf = max_c cs ; e2 = exp(cs)
                cs = work.tile([P, RT, C], f32, tag="cs")
                nc.vector.tensor_tensor(
                    cs[:], cst[:],
                    rxn[:].unsqueeze(2).broadcast_to([P, RT, C]), Alu.mult)
            conf = small.tile([P, RT], f32, tag="conf")
            nc.vector.tensor_reduce(conf[:], cs[:], Ax.X, Alu.max)
            e2 = work.tile([P, RT, C], f32, tag="e2")
            nc.scalar.activation(e2[:], cs[:], Act.Exp)
            s2 = small.tile([P, RT], f32, tag="s2")
            nc.vector.tensor_reduce(s2[:], e2[:], Ax.X, Alu.add)

            # softmin over classes: e1 = exp(-(l1 - min)), s1 = sum
            m = small.tile([P, RT], f32, tag="m")
            nc.vector.tensor_reduce(m[:], l1[:], Ax.X, Alu.min)
            sh = work.tile([P, RT, C], f32, tag="sh")
            nc.vector.tensor_tensor(
                sh[:], l1[:], m[:].unsqueeze(2).broadcast_to([P, RT, C]),
                Alu.subtract)
            e1 = work.tile([P, RT, C], f32, tag="e1")
            nc.scalar.activation(e1[:], sh[:], Act.Exp, scale=-1.0)
            s1 = small.tile([P, RT], f32, tag="s1")
            nc.vector.tensor_reduce(s1[:], e1[:], Ax.X, Alu.add)

            # out = (e1*e2) * (conf / (s1*s2))
            den = small.tile([P, RT], f32, tag="den")
            nc.vector.tensor_tensor(den[:], s1[:], s2[:], Alu.mult)
            rden = small.tile([P, RT], f32, tag="rden")
            nc.vector.reciprocal(rden[:], den[:])
            fac = small.tile([P, RT], f32, tag="fac")
            nc.vector.tensor_tensor(fac[:], conf[:], rden[:], Alu.mult)
            prod = work.tile([P, RT, C], f32, tag="prod")
            nc.vector.tensor_tensor(prod[:], e1[:], e2[:], Alu.mult)
            outt = work.tile([P, RT, C], f32, tag="outt")
            nc.vector.tensor_tensor(
                outt[:], prod[:],
                fac[:].unsqueeze(2).broadcast_to([P, RT, C]), Alu.mult)
            nc.sync.dma_start(
                Yd[:].rearrange("(k p) c -> p k c", p=P), outt[:])

    _split_excess_waits(nc)
    return nc


def prep_in_maps(X: np.ndarray, grp: np.ndarray):
    """Host-side sharding + layout prep (per the data-parallel hint)."""
    X16 = np.ascontiguousarray(X, dtype=np.float32).astype(np.float16)
    g = np.ascontiguousarray(grp.reshape(D, C), dtype=np.float32)
    GD16 = g.astype(np.float16)
    GT32 = np.ascontiguousarray(g.T)
    GR16 = np.ascontiguousarray(g.T.astype(np.float16)).reshape(1, C * D)
    in_maps = []
    for s in range(N_CORES):
        xs = X16[s * R:(s + 1) * R]
        in_maps.append({
            "X16": xs,
            "XT16": np.ascontiguousarray(xs.T),
            "GD16": GD16,
            "GT32": GT32,
            "GR16": GR16,
        })
    return in_maps


def kernel(X: np.ndarray, grp: np.ndarray) -> np.ndarray:
    from concourse.bass_utils import run_bass_kernel_spmd

    if "nc" not in _CACHE:
        _CACHE["nc"] = _build_nc()
    nc = _CACHE["nc"]

    in_maps = prep_in_maps(X, grp)
    last_err = None
    for _attempt in range(3):
        try:
            res = run_bass_kernel_spmd(nc, in_maps, list(range(N_CORES)))
            break
        except Exception as e:  # transient device/tunnel hiccups
            last_err = e
            import time
            time.sleep(2.0)
    else:
        raise last_err
    out = np.concatenate(
        [res.results[i]["Y"] for i in range(N_CORES)], axis=0
    )
    return np.ascontiguousarray(out, dtype=np.float32)


# revision 6
# speedup vs baseline: 89.9221x; 1.2941x over previous
"""Trainium2 Bass kernel for the vq_codebook classifier problem.

Computes, for X [4096, 512] f32 and grp [1, 512, 100] f32:
    l1   = sum_d |X[n,d] - grp[0,d,c]|             -> [N, C]
    norm = softmax(-l1, axis=1)
    cs   = (X @ g) / max(|X| * |g|, eps)           (cosine similarity)
    out  = max_c(cs) * softmax(cs, axis=1) * norm

Sharding: data-parallel over N across 8 NeuronCores (512 rows each),
grp replicated.

This environment pays a large fixed cost PER INSTRUCTION (measured
~30-50us each, regardless of the work an instruction does), so the
kernel is built around a minimal instruction count with huge access
patterns:

  * All dense inputs (X row-tiled, X^T, g d-tiled, g^T, an f32
    identity) are packed host-side into ONE DRAM tensor in the exact
    SBUF layout and loaded by ONE DMA; views (incl. an f32 bitcast for
    the identity) carve it up.  Host-side packing/transposition is
    layout-only prep, part of the sharding contract; all real math
    (norms, distances, GEMM, softmaxes) runs on device.
  * The L1-distance tensor |x[n,d] - g[d,c]| for a block of 25 classes
    x all 512 rows is ONE vector-engine tensor_tensor subtract over a
    [128, 4, 25, 512] access pattern (x broadcast over the class dim
    with a stride-0 AP; the c-major prototype table - replicated to all
    128 partitions by a partition-broadcast DMA straight from DRAM -
    broadcast over the row-tile dim).  The d-reduction with |.| is ONE
    tensor_reduce(axis=X, apply_absolute_value=True).  12 instructions
    for the whole 26M-element L1 computation.
  * The cosine GEMM runs transposed ([c, n] = g16-stationary x
    X^T-moving, 4 matmuls) and is transposed back to [n, c] by 4 PE
    transposes; 1/|g| is folded in before the transpose, 1/|x| after,
    each as one big broadcasted tensor_tensor.
  * The epilogue is fused across all 4 row-tiles with [128, 4, 100]
    APs.  The softmin shift uses a per-partition (XY) min so it rides
    the activation bias port (softmax ratios are invariant to any
    per-row constant, and each row lives on one partition, so a
    per-partition constant is exact); e1/e2 share one [128, 8, 100]
    tile so one reduce yields both softmax denominators; both sqrt and
    both reciprocal calls are packed into single [128, 5] ops.
"""

import numpy as np

P = 128
R = 512          # rows per core (4096 / 8 cores)
D = 512
C = 100
RT = R // P      # 4 row-tiles
NT = D // P      # 4 d-tiles
CB = 25          # classes per L1 block
NCB = C // CB    # 4 class blocks
N_CORES = 8

# packed input layout (f16 columns per partition)
_OX = 0                    # x16   [128, 4, 512]
_OXT = _OX + RT * D        # xt16  [128, 4, 512]
_OG = _OXT + NT * R        # g16   [128, 4, 100]
_OGT = _OG + NT * C        # gt16  [100, 512] (partitions 0..99)
_OID = _OGT + D            # ident [128, 128] f32 (bitcast, 256 f16 cols)
_IN_W = _OID + 2 * P

_CACHE = {}


def _split_excess_waits(nc, limit=1):
    """walrus in this container rejects instructions carrying more than
    one sync wait ("Too many sync wait commands"). Hoist excess waits
    onto same-engine NoOps inserted immediately before the instruction."""
    import concourse.mybir as mb
    import bass_rust

    n_id = [0]

    def mknop(engine, waits):
        n_id[0] += 1
        return bass_rust.InstNoOp(
            name=f"waitsplit-{n_id[0]}", engine=engine, ins=[], outs=[],
            sync_info=mb.SyncInfo(on_wait=list(waits), on_update=[]),
        )

    for fn in nc.m.functions:
        for bb in fn.blocks:
            insts = bb.instructions
            out = []
            for inst in insts:
                si = inst.sync_info
                if si is not None and si.on_wait and len(si.on_wait) > limit:
                    waits = list(si.on_wait)
                    extra, keep = waits[:-limit], waits[-limit:]
                    for w in extra:
                        out.append(mknop(inst.engine, [w]))
                    inst.sync_info = mb.SyncInfo(
                        on_wait=keep, on_update=list(si.on_update)
                    )
                out.append(inst)
            insts[:] = out
    return nc


def _build_nc(reps: int = 1):
    import concourse.bass as bass
    import concourse.mybir as mybir
    import concourse.tile as tile
    from contextlib import ExitStack

    f32 = mybir.dt.float32
    f16 = mybir.dt.float16
    Alu = mybir.AluOpType
    Act = mybir.ActivationFunctionType
    Ax = mybir.AxisListType

    nc = bass.Bass(target_bir_lowering=False)
    INd = nc.declare_dram_parameter("IN16", [P, _IN_W], f16, isOutput=False)
    GR16d = nc.declare_dram_parameter("GR16", [1, C * D], f16, isOutput=False)
    Yd = nc.declare_dram_parameter("Y", [R, C], f32, isOutput=True)

    with ExitStack() as ctx:
        tc = ctx.enter_context(tile.TileContext(nc))
        inp = ctx.enter_context(tc.tile_pool(name="inp", bufs=1))
        grep_pool = ctx.enter_context(tc.tile_pool(name="grep", bufs=2))
        dpool = ctx.enter_context(tc.tile_pool(name="dpool", bufs=1))
        work = ctx.enter_context(tc.tile_pool(name="work", bufs=1))
        small = ctx.enter_context(tc.tile_pool(name="small", bufs=2))

        for _rep in range(reps):
            # ---- one DMA for all dense inputs ----
            big = inp.tile([P, _IN_W], f16, tag="in")
            nc.sync.dma_start(big[:], INd[:])
            x16 = big[:, _OX:_OX + RT * D].rearrange("p (k d) -> p k d", k=RT)
            xt16 = big[:, _OXT:_OXT + NT * R].rearrange(
                "p (t n) -> p t n", t=NT)
            g16 = big[:, _OG:_OG + NT * C].rearrange("p (t c) -> p t c", t=NT)
            gt16 = big[0:C, _OGT:_OGT + D]
            ident = big[:, _OID:_OID + 2 * P].bitcast(f32)

            # ---- norms: rxn = 1/|x| [128, 4]; rgn = 1/|g| [100, 1] ----
            # (sqrt+reciprocal for both packed into single [128, 5] ops)
            xsq = dpool.tile([P, RT, D], f32, tag="diff", name="xsq")
            nc.vector.tensor_tensor(xsq[:], x16, x16, Alu.mult)
            nrm2 = small.tile([P, RT + 1], f32, tag="nrm2")
            nc.vector.tensor_reduce(nrm2[:, 0:RT], xsq[:], Ax.X, Alu.add)
            junk = dpool.tile([C, D], f32, tag="diff", name="junk")
            nc.scalar.activation(junk[:], gt16, Act.Square,
                                 accum_out=nrm2[0:C, RT:RT + 1])
            nrm = small.tile([P, RT + 1], f32, tag="nrm")
            nc.scalar.activation(nrm[:], nrm2[:], Act.Sqrt)
            rall = small.tile([P, RT + 1], f32, tag="rall")
            nc.vector.reciprocal(rall[:], nrm[:])
            rxn = rall[:, 0:RT]
            rgn = rall[0:C, RT:RT + 1]

            # ---- cosine GEMM, transposed: dotT[c, n] ----
            with tc.tile_pool(name="ps", bufs=2, space="PSUM") as psp:
                dotT = psp.tile([C, R], f32, tag="dotT")
                for t in range(NT):
                    nc.tensor.matmul(
                        dotT[:], lhsT=g16[:, t, :], rhs=xt16[:, t, :],
                        start=(t == 0), stop=(t == NT - 1))
                csT = work.tile([C, R], f32, tag="csT")
                nc.vector.tensor_tensor(
                    csT[:], dotT[:], rgn.broadcast_to([C, R]), Alu.mult)
                # transpose back to [n, c] layout: cst [128, RT, C] (PSUM)
                cst = psp.tile([P, RT, C], f32, tag="cst")
                for k in range(RT):
                    nc.tensor.transpose(
                        cst[:, k, :], csT[:, k * P:(k + 1) * P],
                        ident[0:C, 0:C])

                # ---- L1 distances: l1[n-part, k, c] ----
                l1 = work.tile([P, RT, C], f32, tag="l1")
                for cb in range(NCB):
                    grepb = grep_pool.tile([P, CB * D], f16, tag="grep")
                    nc.sync.dma_start(
                        grepb[:],
                        GR16d[0:1, cb * CB * D:(cb + 1) * CB * D]
                        .partition_broadcast(P))
                    diff = dpool.tile([P, RT, CB, D], f16, tag="diff")
                    nc.vector.tensor_tensor(
                        diff[:],
                        x16.unsqueeze(2).broadcast_to([P, RT, CB, D]),
                        grepb[:].rearrange("p (c d) -> p c d", c=CB)
                        .unsqueeze(1).broadcast_to([P, RT, CB, D]),
                        Alu.subtract)
                    nc.vector.tensor_reduce(
                        l1[:, :, cb * CB:(cb + 1) * CB], diff[:], Ax.X,
                        Alu.add, apply_absolute_value=True)

                # ---- epilogue, fused over all row-tiles ----
                cs = work.tile([P, RT, C], f32, tag="cs")
                nc.vector.tensor_tensor(
                    cs[:], cst[:],
                    rxn.unsqueeze(2).broadcast_to([P, RT, C]), Alu.mult)
            conf = small.tile([P, RT], f32, tag="conf")
            nc.vector.tensor_reduce(conf[:], cs[:], Ax.X, Alu.max)

            # e2 = exp(cs) (|cs|<=1, no shift); e1 = exp(-(l1 - m2)) with a
            # per-partition min shift (exact: softmax ratios are invariant
            # to per-row constants and rows live on single partitions)
            e12 = work.tile([P, 2 * RT, C], f32, tag="e12")
            nc.scalar.activation(e12[:, 0:RT, :], cs[:], Act.Exp)
            m2 = small.tile([P, 1], f32, tag="m2")
            nc.vector.tensor_reduce(m2[:], l1[:], Ax.XY, Alu.min)
            nc.scalar.activation(e12[:, RT:2 * RT, :], l1[:], Act.Exp,
                                 bias=m2[:], scale=-1.0)
            s12 = small.tile([P, 2 * RT], f32, tag="s12")
            nc.vector.tensor_reduce(s12[:], e12[:], Ax.X, Alu.add)

            # out = (e1*e2) * (conf / (s1*s2))
            den = small.tile([P, RT], f32, tag="den")
            nc.vector.tensor_tensor(
                den[:], s12[:, RT:2 * RT], s12[:, 0:RT], Alu.mult)
            rden = small.tile([P, RT], f32, tag="rden")
            nc.vector.reciprocal(rden[:], den[:])
            fac = small.tile([P, RT], f32, tag="fac")
            nc.vector.tensor_tensor(fac[:], conf[:], rden[:], Alu.mult)
            prod = work.tile([P, RT, C], f32, tag="prod")
            nc.vector.tensor_tensor(
                prod[:], e12[:, RT:2 * RT, :], e12[:, 0:RT, :], Alu.mult)
            outt = work.tile([P, RT, C], f32, tag="outt")
            nc.vector.tensor_tensor(
                outt[:], prod[:],
                fac[:].unsqueeze(2).broadcast_to([P, RT, C]), Alu.mult)
            nc.sync.dma_start(
                Yd[:].rearrange("(k p) c -> p k c", p=P), outt[:])

    _split_excess_waits(nc)
    return nc


def prep_in_maps(X: np.ndarray, grp: np.ndarray):
    """Host-side sharding + layout prep (per the data-parallel hint)."""
    X16 = np.ascontiguousarray(X, dtype=np.float32).astype(np.float16)
    g = np.ascontiguousarray(grp.reshape(D, C), dtype=np.float32)
    G16 = g.astype(np.float16)
    GT16 = np.ascontiguousarray(G16.T)                    # [100, 512]
    GR16 = GT16.reshape(1, C * D)                         # c-major flat
    ident = np.eye(P, dtype=np.float32).view(np.float16)  # [128, 256]

    in_maps = []
    for s in range(N_CORES):
        xs = X16[s * R:(s + 1) * R]
        IN = np.zeros((P, _IN_W), dtype=np.float16)
        IN[:, _OX:_OX + RT * D] = (
            xs.reshape(RT, P, D).transpose(1, 0, 2).reshape(P, RT * D))
        IN[:, _OXT:_OXT + NT * R] = (
            np.ascontiguousarray(xs.T).reshape(NT, P, R)
            .transpose(1, 0, 2).reshape(P, NT * R))
        IN[:, _OG:_OG + NT * C] = (
            G16.reshape(NT, P, C).transpose(1, 0, 2).reshape(P, NT * C))
        IN[0:C, _OGT:_OGT + D] = GT16
        IN[:, _OID:_OID + 2 * P] = ident
        in_maps.append({"IN16": IN, "GR16": GR16})
    return in_maps


def kernel(X: np.ndarray, grp: np.ndarray) -> np.ndarray:
    from concourse.bass_utils import run_bass_kernel_spmd

    if "nc" not in _CACHE:
        _CACHE["nc"] = _build_nc()
    nc = _CACHE["nc"]

    in_maps = prep_in_maps(X, grp)
    last_err = None
    for _attempt in range(3):
        try:
            res = run_bass_kernel_spmd(nc, in_maps, list(range(N_CORES)))
            break
        except Exception as e:  # transient device/tunnel hiccups
            last_err = e
            import time
            time.sleep(2.0)
    else:
        raise last_err
    out = np.concatenate(
        [res.results[i]["Y"] for i in range(N_CORES)], axis=0
    )
    return np.ascontiguousarray(out, dtype=np.float32)
